# revision 6
# baseline (speedup 1.0000x reference)
"""Trainium2 Bass kernel for nn_Downsample_Spa: sigma-conv + gaussian unfold downsample.

Math (per batch image, one NeuronCore each; batch of 8 -> 8 cores):
  xp = reflect_pad(x)                                  # [64,130,130]
  sigma[o,p] = clamp(BN(conv3x3(xp))[o,p], 1e-4)       # at stride-2 positions p only
  graw[o,p]  = exp(-0.5*d2[o]/sigma^2 - ln64) / sigma  # /64 guards fp16 range; cancels in the ratio
  out[c,p]   = sum_o graw[o,p]*xp[c,p+off(o)] / sum_o graw[o,p]

v2 design (vs v1 baseline):
 - partitions = (row-half hh, channel c) = 128; host pre-pads (reflect), fp16, and
   parity-splits columns into TWO planes (w=2j / w=2j+1); the former third plane
   (w=2j+2) is just plane0 shifted one j-slot, so taps read plane0 at col+1.
   Input: 2.2MB/core (was 3.3MB).
 - conv: 9 accumulating fp16 matmuls per 512-position block (block-diagonal weights,
   M=18 = both row halves). sigma [18,512] in PSUM, per-BLOCK g pipeline:
   clamp (DVE tensor_scalar), fast reciprocal, ACT Square+Exp, mult -> gb bf16.
 - unfold: one-hot bf16 matmuls broadcast gb to the 128 (hh,c) partitions, 3 taps
   per PSUM rowgroup + a Srep (sum) matmul; ACT copies rowgroups to fp16 gcAll;
   all 9 tap products fp16 on DVE; pair tree ytA+ytB -> t4 -> t2 -> (gpsimd) t1 ->
   +center -> *1/S -> fp16 out DMA per block. Host converts fp32.
 - PE kept HAM-warm: ~3.4us warmup burst, per-block [Srep, 9 bcasts, next conv]
   bursts, LDWEIGHTS fillers in gaps (cold PE halves matmul throughput).
 - DMA: input chunks issued from 4 different engines in parallel; fp16 output.
"""

import os
import sys

import numpy as np

if "/opt/trn_rl_repo" not in sys.path:
    sys.path.insert(0, "/opt/trn_rl_repo")

K = 3
BN_EPS = 1e-5
SIGMA_MIN = 1e-4
GSCALE_LN = float(np.log(64.0))   # graw scaled by 1/64 (folded into exp bias)
N, C, H, W = 8, 64, 128, 128
HO = WO = 64
HH = 2
RS = 65                  # padded-row slots per partition-half
HOC = 32
NBLK = 4
BR = HOC // NBLK         # 8 output rows per block
NPOS = BR * WO           # 512
PL = 2                   # x col-parity planes: w=2j / w=2j+1
JW = 66                  # j slots per plane (65 used, 66 for alignment)
CR = 17                  # rows per DMA chunk tile (16 + 1 overlap)

# f32 consts tensor columns
_D2 = 0                  # -0.5*d2[o] per (hh,o)
_BC = 1                  # bn_bias - sigma_min
_LB = 2                  # exp bias: constant -ln(64) per partition
_NCC = 3

_STATE = {}


def _build_consts(conv_w, bn_gamma, bn_beta, bn_mean, bn_var):
    s = (bn_gamma / np.sqrt(bn_var + BN_EPS)).astype(np.float32)
    wf = conv_w.astype(np.float32) * s[:, None, None, None]           # [9,64,3,3]
    bias = (bn_beta - bn_mean * s).astype(np.float32)

    cst = np.zeros((18, _NCC), np.float32)
    d2 = np.array([(kk // 3 - 1) ** 2 + (kk % 3 - 1) ** 2 for kk in range(9)], np.float32)
    for hh in range(HH):
        cst[hh * 9:hh * 9 + 9, _D2] = -0.5 * d2
        cst[hh * 9:hh * 9 + 9, _BC] = bias - SIGMA_MIN
        cst[hh * 9:hh * 9 + 9, _LB] = -GSCALE_LN

    # conv weights, block-diagonal per tap: win[k=hh*64+c, tap*18 + hh*9+o]
    win = np.zeros((128, 9 * 18), np.float16)
    for tap in range(9):
        i, j = tap // 3, tap % 3
        for hh in range(HH):
            win[hh * 64:hh * 64 + 64, tap * 18 + hh * 9:tap * 18 + hh * 9 + 9] = \
                wf[:, :, i, j].T.astype(np.float16)

    # one-hot / ones broadcast weights: gin[k=hh*9+o, tap*128 + hh*64+c]
    import ml_dtypes
    gin = np.zeros((18, 10 * 128), ml_dtypes.bfloat16)
    for hh in range(HH):
        gin[hh * 9:hh * 9 + 9, 9 * 128 + hh * 64:9 * 128 + hh * 64 + 64] = 1.0
        for tap in range(9):
            gin[hh * 9 + tap, tap * 128 + hh * 64:tap * 128 + hh * 64 + 64] = 1.0
    return cst, win, gin


def _build_bass(for_sim=False):
    import concourse.bass as bass
    import concourse.tile as tile
    from concourse import mybir

    f32 = mybir.dt.float32
    f16 = mybir.dt.float16
    bf16 = mybir.dt.bfloat16
    MULT = mybir.AluOpType.mult
    ADD = mybir.AluOpType.add
    MAX = mybir.AluOpType.max
    AF = mybir.ActivationFunctionType

    if for_sim:
        nc = bass.Bass("TRN2", target_bir_lowering=False, detect_race_conditions=False)
    else:
        from concourse import bacc
        nc = bacc.Bacc()
    xin = nc.dram_tensor("xin", [128, RS, PL, JW], f16, kind="ExternalInput")
    cin = nc.dram_tensor("cin", [18, _NCC], f32, kind="ExternalInput")
    win = nc.dram_tensor("win", [128, 9 * 18], f16, kind="ExternalInput")
    gin = nc.dram_tensor("gin", [18, 10 * 128], bf16, kind="ExternalInput")
    out = nc.dram_tensor("out", [128, HOC, WO], f16, kind="ExternalOutput")

    with tile.TileContext(nc) as tc:
        from contextlib import ExitStack
        with ExitStack() as ctx:
            big = ctx.enter_context(tc.tile_pool(name="big", bufs=1))
            gsb = ctx.enter_context(tc.tile_pool(name="gsb", bufs=3))
            y_p = ctx.enter_context(tc.tile_pool(name="y", bufs=2))
            ps_s = ctx.enter_context(tc.tile_pool(name="ps_s", bufs=2, space="PSUM"))
            ps_g = ctx.enter_context(tc.tile_pool(name="ps_g", bufs=2, space="PSUM"))

            # ---- input DMAs: descriptors issued from different engines in parallel
            ws = big.tile([128, 9 * 18], f16)
            nc.gpsimd.dma_start(out=ws[:], in_=win[:])
            xsk = []
            for blk in range(NBLK):
                xs = big.tile([128, CR, PL, JW], f16, tag=f"xs{blk}")
                xsk.append(xs)
            nc.sync.dma_start(out=xsk[0][:], in_=xin[:, 0:CR, :, :])
            nc.scalar.dma_start(out=xsk[1][:], in_=xin[:, 16:16 + CR, :, :])
            nc.sync.dma_start(out=xsk[2][:], in_=xin[:, 32:32 + CR, :, :])
            cs = big.tile([18, _NCC], f32)
            nc.gpsimd.dma_start(out=cs[:], in_=cin[:])
            gs = big.tile([18, 10 * 128], bf16)
            nc.gpsimd.dma_start(out=gs[:], in_=gin[:])
            nc.scalar.dma_start(out=xsk[3][:], in_=xin[:, 48:48 + CR, :, :])

            def xtap(tap, blk):
                # [128, 8, 64] fp16 step-1 view for conv tap (i,j) in block blk
                i, j = tap // 3, tap % 3
                if j < 2:
                    return xsk[blk][:, i:i + 2 * BR - 1:2, j, 0:WO]
                return xsk[blk][:, i:i + 2 * BR - 1:2, 0, 1:WO + 1]

            def xpair(i, blk):
                # [128, 2, 8, 64] planes (w=2j, 2j+1) for tap row i
                return xsk[blk][:, i:i + 2 * BR - 1:2, 0:PL, 0:WO].transpose([0, 2, 1, 3])

            # ---- PE warm-up: ~3.4us of matmuls on the (early-arriving) weights tile
            # while input DMA lands, so HAM un-throttles before the conv ----
            wu = ps_s.tile([18, BR, WO], f32, tag="sig")
            for _ in range(24):
                nc.tensor.matmul(wu[:, 0:3, 0:54], ws[:, 0:18], ws[:, 0:162],
                                 start=True, stop=True)

            def ldw_fill(n):
                # LDWEIGHTS fillers: keep the PE array active through dependency
                # gaps so HAM holds K=8/8 (cold PE runs matmuls at half rate)
                for _ in range(n):
                    nc.tensor.ldweights(ws[:, 0:128])

            def conv_emit(blk):
                sig = ps_s.tile([18, BR, WO], f32, tag="sig")
                for tap in range(9):
                    nc.tensor.matmul(
                        sig[:],
                        ws[:, tap * 18:(tap + 1) * 18],
                        xtap(tap, blk),
                        start=(tap == 0), stop=(tap == 8),
                    )
                sc = gsb.tile([18, BR, WO], f32, tag="sc")
                nc.vector.tensor_scalar(out=sc[:], in0=sig[:],
                                        scalar1=cs[:, _BC:_BC + 1],
                                        scalar2=float(SIGMA_MIN),
                                        op0=ADD, op1=MAX)
                return sc

            def ginv_emit(sc):
                inv = gsb.tile([18, BR, WO], f32, tag="inv")
                nc.vector.reciprocal_approx_fast(out=inv[:], in_=sc[:])
                qt = gsb.tile([18, BR, WO], f32, tag="qt")
                nc.scalar.activation(out=qt[:], in_=inv[:], func=AF.Square)
                et = gsb.tile([18, BR, WO], f32, tag="et")
                nc.scalar.activation(out=et[:], in_=qt[:], func=AF.Exp,
                                     scale=cs[:, _D2:_D2 + 1],
                                     bias=cs[:, _LB:_LB + 1])
                return inv, et

            def gb_emit(inv, et):
                gb = gsb.tile([18, BR, WO], bf16, tag="gb")
                nc.vector.tensor_tensor(out=gb[:], in0=et[:], in1=inv[:], op=MULT)
                return gb

            def burst_emit(blk, gb, next_conv_blk):
                # PE burst: Srep + 9 broadcast matmuls (+ next conv block),
                # back-to-back so each block re-triggers the HAM warm window.
                gbf = gb[:]  # [18, 8, 64]; matmul checks free_size only
                Srep = ps_s.tile([128, BR, WO], f32, tag="sig")
                nc.tensor.matmul(Srep[:], gs[:, 9 * 128:10 * 128], gbf,
                                 start=True, stop=True)
                groups = []
                for g in range(3):  # rowgroups: taps (0,1,2), (3,4,5), (6,7,8)
                    g3 = ps_g.tile([128, 3, BR, WO], f32, tag="grep")
                    for k in range(3):
                        tap = 3 * g + k
                        nc.tensor.matmul(g3[:, k], gs[:, tap * 128:(tap + 1) * 128],
                                         gbf, start=True, stop=True)
                    groups.append(g3)
                    ldw_fill(2)
                sc_next = None
                if next_conv_blk is not None:
                    sc_next = conv_emit(next_conv_blk)
                else:
                    ldw_fill(8)
                return Srep, groups, sc_next

            def unfold_emit(blk, Srep, groups, late):
                # ACT: copy broadcast g to fp16 (one big tile, slots = tap order)
                gc = y_p.tile([128, 9, BR, WO], f16, tag="gc")
                for g in range(3):
                    nc.scalar.activation(out=gc[:, 3 * g:3 * g + 3],
                                         in_=groups[g][:], func=AF.Copy)

                rr = y_p.tile([128, BR, WO], f32, tag="rr")
                nc.vector.reciprocal_approx_fast(out=rr[:], in_=Srep[:])

                # products, all fp16: ytA = [r0j0, r0j1, r0j2, r1j0],
                # ytB = [r2j0, r2j1, r2j2, r1j2], yC = center
                ytA = y_p.tile([128, 4, BR, WO], f16, tag="ytA")
                ytB = y_p.tile([128, 4, BR, WO], f16, tag="ytB")
                yC = y_p.tile([128, BR, WO], f16, tag="yC")
                nc.vector.tensor_tensor(out=ytA[:, 0:2], in0=xpair(0, blk),
                                        in1=gc[:, 0:2], op=MULT)
                nc.vector.tensor_tensor(out=ytA[:, 2], in0=xtap(2, blk),
                                        in1=gc[:, 2], op=MULT)
                nc.vector.tensor_tensor(out=ytA[:, 3], in0=xtap(3, blk),
                                        in1=gc[:, 3], op=MULT)
                nc.vector.tensor_tensor(out=yC[:], in0=xtap(4, blk),
                                        in1=gc[:, 4], op=MULT)
                nc.vector.tensor_tensor(out=ytB[:, 3], in0=xtap(5, blk),
                                        in1=gc[:, 5], op=MULT)
                nc.vector.tensor_tensor(out=ytB[:, 0:2], in0=xpair(2, blk),
                                        in1=gc[:, 6:8], op=MULT)
                nc.vector.tensor_tensor(out=ytB[:, 2], in0=xtap(8, blk),
                                        in1=gc[:, 8], op=MULT)

                # pair tree (fp16) + center + normalize; tail offloaded to gpsimd
                t4 = y_p.tile([128, 4, BR, WO], f16, tag="t4")
                nc.vector.tensor_tensor(out=t4[:], in0=ytA[:], in1=ytB[:], op=ADD)
                t2 = y_p.tile([128, 2, BR, WO], f16, tag="t2")
                nc.vector.tensor_tensor(out=t2[:], in0=t4[:, 0:2], in1=t4[:, 2:4], op=ADD)
                eng1 = nc.vector if late else nc.gpsimd
                t1 = y_p.tile([128, BR, WO], f16, tag="t1")
                eng1.tensor_tensor(out=t1[:], in0=t2[:, 0], in1=t2[:, 1], op=ADD)
                tC = y_p.tile([128, BR, WO], f16, tag="tC")
                eng1.tensor_tensor(out=tC[:], in0=t1[:], in1=yC[:], op=ADD)
                o16 = y_p.tile([128, BR, WO], f16, tag="o16")
                eng1.tensor_tensor(out=o16[:], in0=tC[:], in1=rr[:], op=MULT)
                nc.sync.dma_start(out=out[:, BR * blk:BR * (blk + 1), :], in_=o16[:])

            # ---- schedule ----
            sc0 = conv_emit(0)
            inv0, et0 = ginv_emit(sc0)
            gb0 = gb_emit(inv0, et0)
            sc1 = conv_emit(1)
            inv1, et1 = ginv_emit(sc1)
            gb1 = gb_emit(inv1, et1)

            gbs = [gb0, gb1, None, None]
            scs = [None, None, None, None]
            for blk in range(NBLK):
                nxt = blk + 2 if blk + 2 < NBLK else None
                Srep, groups, sc_next = burst_emit(blk, gbs[blk], nxt)
                if nxt is not None:
                    scs[nxt] = sc_next
                    invn, etn = ginv_emit(sc_next)
                    gbs[nxt] = gb_emit(invn, etn)
                unfold_emit(blk, Srep, groups, late=(blk == NBLK - 1))

    if not for_sim and not nc.is_finalized():
        nc.finalize()
    return nc


def _prep_inputs(x, conv_w, bn_gamma, bn_beta, bn_mean, bn_var):
    cst, win, gin = _build_consts(conv_w, bn_gamma, bn_beta, bn_mean, bn_var)
    xp = np.pad(np.asarray(x, np.float32), ((0, 0), (0, 0), (1, 1), (1, 1)),
                mode="reflect").astype(np.float16)                    # [8,64,130,130]
    in_maps = []
    for n in range(N):
        xc = np.concatenate([xp[n, :, 0:RS, :], xp[n, :, 64:64 + RS, :]], axis=0)
        xpl = np.zeros((128, RS, PL, JW), np.float16)
        xpl[:, :, 0, 0:65] = xc[:, :, 0:130:2]
        xpl[:, :, 1, 0:65] = xc[:, :, 1:130:2]
        in_maps.append({"xin": xpl, "cin": cst, "win": win, "gin": gin})
    return in_maps


def _gather(results):
    out = np.empty((N, C, HO, WO), np.float32)
    for n in range(N):
        d = np.asarray(results[n]["out"], np.float32)
        out[n, :, 0:HOC, :] = d[0:64]
        out[n, :, HOC:, :] = d[64:128]
    return out


def _enable_axon_trace():
    """Register the NTFF profile hook that this image's antenv lacks."""
    if _STATE.get("trace_hooked"):
        return
    import types
    import antenv
    from concourse import bass_utils
    mod = types.ModuleType("antenv.axon_hooks")
    mod._hook = None
    mod.set_axon_ntff_profile_hook = lambda h: setattr(mod, "_hook", h)
    mod.get_axon_ntff_profile_hook = lambda: mod._hook
    sys.modules["antenv.axon_hooks"] = mod
    antenv.axon_hooks = mod
    from trn_agent_boot.trn_boot import _ntff_profile_via_ctypes
    mod._hook = _ntff_profile_via_ctypes("/opt/axon/libaxon_pjrt.so")
    bass_utils.upload_artifacts = lambda tmpdir: tmpdir
    _STATE["trace_hooked"] = True


def run(x, conv_w, bn_gamma, bn_beta, bn_mean, bn_var, trace=False):
    from concourse.bass_utils import run_bass_kernel_spmd
    if trace:
        _enable_axon_trace()
    if "nc" not in _STATE:
        _STATE["nc"] = _build_bass()
    in_maps = _prep_inputs(x, conv_w, bn_gamma, bn_beta, bn_mean, bn_var)
    res = run_bass_kernel_spmd(_STATE["nc"], in_maps, list(range(N)), trace=trace)
    _STATE["last"] = res
    return _gather(res.results)


def kernel(x, conv_w, bn_gamma, bn_beta, bn_mean, bn_var):
    return run(x, conv_w, bn_gamma, bn_beta, bn_mean, bn_var,
               trace=bool(int(os.environ.get("KERNEL_TRACE", "0"))))


# revision 9
# speedup vs baseline: 1.0035x; 1.0035x over previous
"""Trainium2 Bass kernel for nn_Downsample_Spa: sigma-conv + gaussian unfold downsample.

Math (per batch image, one NeuronCore each; batch of 8 -> 8 cores):
  xp = reflect_pad(x)                                  # [64,130,130]
  sigma[o,p] = clamp(BN(conv3x3(xp))[o,p], 1e-4)       # at stride-2 positions p only
  graw[o,p]  = exp(-0.5*d2[o]/sigma^2 - ln64) / sigma  # /64 guards fp16 range; cancels in the ratio
  out[c,p]   = sum_o graw[o,p]*xp[c,p+off(o)] / sum_o graw[o,p]

v2 design (vs v1 baseline):
 - partitions = (row-half hh, channel c) = 128; host pre-pads (reflect), fp16, and
   parity-splits columns into TWO planes (w=2j / w=2j+1); the former third plane
   (w=2j+2) is just plane0 shifted one j-slot, so taps read plane0 at col+1.
   Input: 2.2MB/core (was 3.3MB).
 - conv: 9 accumulating fp16 matmuls per 512-position block (block-diagonal weights,
   M=18 = both row halves). sigma [18,512] in PSUM, per-BLOCK g pipeline:
   clamp (DVE tensor_scalar), fast reciprocal, ACT Square+Exp, mult -> gb bf16.
 - unfold: one-hot bf16 matmuls broadcast gb to the 128 (hh,c) partitions, 3 taps
   per PSUM rowgroup + a Srep (sum) matmul; ACT copies rowgroups to fp16 gcAll;
   all 9 tap products fp16 on DVE; pair tree ytA+ytB -> t4 -> t2 -> (gpsimd) t1 ->
   +center -> *1/S -> fp16 out DMA per block. Host converts fp32.
 - PE kept HAM-warm: ~3.4us warmup burst, per-block [Srep, 9 bcasts, next conv]
   bursts, LDWEIGHTS fillers in gaps (cold PE halves matmul throughput).
 - DMA: input chunks issued from 4 different engines in parallel; fp16 output.
"""

import os
import sys

import numpy as np

if "/opt/trn_rl_repo" not in sys.path:
    sys.path.insert(0, "/opt/trn_rl_repo")

K = 3
BN_EPS = 1e-5
SIGMA_MIN = 1e-4
GSCALE_LN = float(np.log(64.0))   # graw scaled by 1/64 (folded into exp bias)
N, C, H, W = 8, 64, 128, 128
HO = WO = 64
HH = 2
RS = 65                  # padded-row slots per partition-half
HOC = 32
NBLK = 4
BR = HOC // NBLK         # 8 output rows per block
NPOS = BR * WO           # 512
PL = 2                   # x col-parity planes: w=2j / w=2j+1
JW = 66                  # j slots per plane (65 used, 66 for alignment)
CR = 17                  # rows per DMA chunk tile (16 + 1 overlap)

# f32 consts tensor columns
_D2 = 0                  # -0.5*d2[o] per (hh,o)
_BC = 1                  # bn_bias - sigma_min
_LB = 2                  # exp bias: constant -ln(64) per partition
_NCC = 3

_STATE = {}


def _build_consts(conv_w, bn_gamma, bn_beta, bn_mean, bn_var):
    s = (bn_gamma / np.sqrt(bn_var + BN_EPS)).astype(np.float32)
    wf = conv_w.astype(np.float32) * s[:, None, None, None]           # [9,64,3,3]
    bias = (bn_beta - bn_mean * s).astype(np.float32)

    cst = np.zeros((18, _NCC), np.float32)
    d2 = np.array([(kk // 3 - 1) ** 2 + (kk % 3 - 1) ** 2 for kk in range(9)], np.float32)
    for hh in range(HH):
        cst[hh * 9:hh * 9 + 9, _D2] = -0.5 * d2
        cst[hh * 9:hh * 9 + 9, _BC] = bias - SIGMA_MIN
        cst[hh * 9:hh * 9 + 9, _LB] = -GSCALE_LN

    # conv weights, block-diagonal per tap: win[k=hh*64+c, tap*18 + hh*9+o]
    win = np.zeros((128, 9 * 18), np.float16)
    for tap in range(9):
        i, j = tap // 3, tap % 3
        for hh in range(HH):
            win[hh * 64:hh * 64 + 64, tap * 18 + hh * 9:tap * 18 + hh * 9 + 9] = \
                wf[:, :, i, j].T.astype(np.float16)

    # one-hot / ones broadcast weights: gin[k=hh*9+o, tap*128 + hh*64+c]
    import ml_dtypes
    gin = np.zeros((18, 10 * 128), ml_dtypes.bfloat16)
    for hh in range(HH):
        gin[hh * 9:hh * 9 + 9, 9 * 128 + hh * 64:9 * 128 + hh * 64 + 64] = 1.0
        for tap in range(9):
            gin[hh * 9 + tap, tap * 128 + hh * 64:tap * 128 + hh * 64 + 64] = 1.0
    return cst, win, gin


def _build_bass(for_sim=False):
    import concourse.bass as bass
    import concourse.tile as tile
    from concourse import mybir

    f32 = mybir.dt.float32
    f16 = mybir.dt.float16
    bf16 = mybir.dt.bfloat16
    MULT = mybir.AluOpType.mult
    ADD = mybir.AluOpType.add
    MAX = mybir.AluOpType.max
    AF = mybir.ActivationFunctionType

    if for_sim:
        nc = bass.Bass("TRN2", target_bir_lowering=False, detect_race_conditions=False)
    else:
        from concourse import bacc
        nc = bacc.Bacc()
    xin = nc.dram_tensor("xin", [128, RS, PL, JW], f16, kind="ExternalInput")
    cin = nc.dram_tensor("cin", [18, _NCC], f32, kind="ExternalInput")
    win = nc.dram_tensor("win", [128, 9 * 18], f16, kind="ExternalInput")
    gin = nc.dram_tensor("gin", [18, 10 * 128], bf16, kind="ExternalInput")
    out = nc.dram_tensor("out", [128, HOC, WO], f16, kind="ExternalOutput")

    with tile.TileContext(nc) as tc:
        from contextlib import ExitStack
        with ExitStack() as ctx:
            big = ctx.enter_context(tc.tile_pool(name="big", bufs=1))
            gsb = ctx.enter_context(tc.tile_pool(name="gsb", bufs=3))
            y_p = ctx.enter_context(tc.tile_pool(name="y", bufs=3))
            ps_s = ctx.enter_context(tc.tile_pool(name="ps_s", bufs=2, space="PSUM"))
            ps_g = ctx.enter_context(tc.tile_pool(name="ps_g", bufs=2, space="PSUM"))

            # warm-up operand: memset scratch, so the PE can start before any DMA
            scr = big.tile([128, 162], f16)
            nc.vector.memset(scr[:], 0.0)

            # ---- input DMAs: descriptors issued from hw queues (sync/scalar) in
            # parallel; block-0 rows split across both queues to land first ----
            ws = big.tile([128, 9 * 18], f16)
            nc.sync.dma_start(out=ws[:], in_=win[:])
            xsk = []
            for blk in range(NBLK):
                xs = big.tile([128, CR, PL, JW], f16, tag=f"xs{blk}")
                xsk.append(xs)
            nc.sync.dma_start(out=xsk[0][:, 0:9], in_=xin[:, 0:9, :, :])
            nc.scalar.dma_start(out=xsk[0][:, 9:CR], in_=xin[:, 9:CR, :, :])
            nc.sync.dma_start(out=xsk[1][:], in_=xin[:, 16:16 + CR, :, :])
            nc.scalar.dma_start(out=xsk[2][:], in_=xin[:, 32:32 + CR, :, :])
            cs = big.tile([18, _NCC], f32)
            nc.gpsimd.dma_start(out=cs[:], in_=cin[:])
            gs = big.tile([18, 10 * 128], bf16)
            nc.gpsimd.dma_start(out=gs[:], in_=gin[:])
            nc.scalar.dma_start(out=xsk[3][:], in_=xin[:, 48:48 + CR, :, :])

            def xtap(tap, blk):
                # [128, 8, 64] fp16 step-1 view for conv tap (i,j) in block blk
                i, j = tap // 3, tap % 3
                if j < 2:
                    return xsk[blk][:, i:i + 2 * BR - 1:2, j, 0:WO]
                return xsk[blk][:, i:i + 2 * BR - 1:2, 0, 1:WO + 1]

            def xpair(i, blk):
                # [128, 2, 8, 64] planes (w=2j, 2j+1) for tap row i
                return xsk[blk][:, i:i + 2 * BR - 1:2, 0:PL, 0:WO].transpose([0, 2, 1, 3])

            # ---- PE warm-up: ~3.4us of matmuls on the memset scratch while input
            # DMA lands, so HAM un-throttles before the conv ----
            wu = ps_s.tile([18, BR, WO], f32, tag="sig")
            for _ in range(26):
                nc.tensor.matmul(wu[:, 0:3, 0:54], scr[:, 0:18], scr[:, 0:162],
                                 start=True, stop=True)

            def ldw_fill(n):
                # LDWEIGHTS fillers: keep the PE array active through dependency
                # gaps so HAM holds K=8/8 (cold PE runs matmuls at half rate)
                for _ in range(n):
                    nc.tensor.ldweights(ws[:, 0:128])

            def conv_mm(blk):
                sig = ps_s.tile([18, BR, WO], f32, tag="sig")
                for tap in range(9):
                    nc.tensor.matmul(
                        sig[:],
                        ws[:, tap * 18:(tap + 1) * 18],
                        xtap(tap, blk),
                        start=(tap == 0), stop=(tap == 8),
                    )
                return sig

            def clamp_emit(sig):
                sc = gsb.tile([18, BR, WO], f32, tag="sc")
                nc.vector.tensor_scalar(out=sc[:], in0=sig[:],
                                        scalar1=cs[:, _BC:_BC + 1],
                                        scalar2=float(SIGMA_MIN),
                                        op0=ADD, op1=MAX)
                return sc

            def inv_emit(sc):
                inv = gsb.tile([18, BR, WO], f32, tag="inv")
                nc.vector.reciprocal_approx_fast(out=inv[:], in_=sc[:])
                return inv

            def qe_emit(inv):
                qt = gsb.tile([18, BR, WO], f32, tag="qt")
                nc.scalar.activation(out=qt[:], in_=inv[:], func=AF.Square)
                et = gsb.tile([18, BR, WO], f32, tag="et")
                nc.scalar.activation(out=et[:], in_=qt[:], func=AF.Exp,
                                     scale=cs[:, _D2:_D2 + 1],
                                     bias=cs[:, _LB:_LB + 1])
                return et

            def gb_emit(inv, et):
                gb = gsb.tile([18, BR, WO], bf16, tag="gb")
                nc.vector.tensor_tensor(out=gb[:], in0=et[:], in1=inv[:], op=MULT)
                return gb

            def burst_mm(blk, gb):
                # PE burst: Srep + 9 broadcast matmuls, back-to-back with the next
                # conv block so each block's PE stretch re-triggers the HAM window.
                gbf = gb[:]  # [18, 8, 64]; matmul checks free_size only
                Srep = ps_s.tile([128, BR, WO], f32, tag="sig")
                nc.tensor.matmul(Srep[:], gs[:, 9 * 128:10 * 128], gbf,
                                 start=True, stop=True)
                groups = []
                for g in range(3):  # rowgroups: taps (0,1,2), (3,4,5), (6,7,8)
                    g3 = ps_g.tile([128, 3, BR, WO], f32, tag="grep")
                    for k in range(3):
                        tap = 3 * g + k
                        nc.tensor.matmul(g3[:, k], gs[:, tap * 128:(tap + 1) * 128],
                                         gbf, start=True, stop=True)
                    groups.append(g3)
                    ldw_fill(2)
                return Srep, groups

            def copies_emit(groups):
                gc = y_p.tile([128, 9, BR, WO], f16, tag="gc")
                for g in range(3):
                    nc.scalar.activation(out=gc[:, 3 * g:3 * g + 3],
                                         in_=groups[g][:], func=AF.Copy)
                return gc

            def products_emit(blk, gc):
                # products, all fp16: ytA = [r0j0, r0j1, r0j2, r1j0],
                # ytB = [r2j0, r2j1, r2j2, r1j2], yC = center
                ytA = y_p.tile([128, 4, BR, WO], f16, tag="ytA")
                ytB = y_p.tile([128, 4, BR, WO], f16, tag="ytB")
                yC = y_p.tile([128, BR, WO], f16, tag="yC")
                nc.vector.tensor_tensor(out=ytA[:, 0:2], in0=xpair(0, blk),
                                        in1=gc[:, 0:2], op=MULT)
                nc.vector.tensor_tensor(out=ytA[:, 2], in0=xtap(2, blk),
                                        in1=gc[:, 2], op=MULT)
                nc.vector.tensor_tensor(out=ytA[:, 3], in0=xtap(3, blk),
                                        in1=gc[:, 3], op=MULT)
                nc.vector.tensor_tensor(out=yC[:], in0=xtap(4, blk),
                                        in1=gc[:, 4], op=MULT)
                nc.vector.tensor_tensor(out=ytB[:, 3], in0=xtap(5, blk),
                                        in1=gc[:, 5], op=MULT)
                nc.vector.tensor_tensor(out=ytB[:, 0:2], in0=xpair(2, blk),
                                        in1=gc[:, 6:8], op=MULT)
                nc.vector.tensor_tensor(out=ytB[:, 2], in0=xtap(8, blk),
                                        in1=gc[:, 8], op=MULT)
                t4 = y_p.tile([128, 4, BR, WO], f16, tag="t4")
                nc.vector.tensor_tensor(out=t4[:], in0=ytA[:], in1=ytB[:], op=ADD)
                return t4, yC

            def tree_tail(blk, t4, yC, rr, late):
                t2 = y_p.tile([128, 2, BR, WO], f16, tag="t2")
                nc.vector.tensor_tensor(out=t2[:], in0=t4[:, 0:2], in1=t4[:, 2:4], op=ADD)
                eng1 = nc.vector if late else nc.gpsimd
                t1 = y_p.tile([128, BR, WO], f16, tag="t1")
                eng1.tensor_tensor(out=t1[:], in0=t2[:, 0], in1=t2[:, 1], op=ADD)
                tC = y_p.tile([128, BR, WO], f16, tag="tC")
                eng1.tensor_tensor(out=tC[:], in0=t1[:], in1=yC[:], op=ADD)
                o16 = y_p.tile([128, BR, WO], f16, tag="o16")
                eng1.tensor_tensor(out=o16[:], in0=tC[:], in1=rr[:], op=MULT)
                nc.sync.dma_start(out=out[:, BR * blk:BR * (blk + 1), :], in_=o16[:])

            # ---- schedule: lead-in (conv0, conv1 + their g-pipes), then per-block
            # [PE burst + next conv] / [copies] / [products] staged for overlap ----
            sig0 = conv_mm(0)
            sc0 = clamp_emit(sig0)
            inv0 = inv_emit(sc0)
            et0 = qe_emit(inv0)
            sig1 = conv_mm(1)
            sc1 = clamp_emit(sig1)
            inv1 = inv_emit(sc1)
            et1 = qe_emit(inv1)
            gbs = [gb_emit(inv0, et0), gb_emit(inv1, et1), None, None]
            ldw_fill(6)

            for blk in range(NBLK):
                nxt = blk + 2 if blk + 2 < NBLK else None
                Srep, groups = burst_mm(blk, gbs[blk])
                rr = y_p.tile([128, BR, WO], f32, tag="rr")
                nc.vector.reciprocal_approx_fast(out=rr[:], in_=Srep[:])
                gc = copies_emit(groups)
                if nxt is not None:
                    sign = conv_mm(nxt)
                    scn = clamp_emit(sign)
                else:
                    ldw_fill(10)
                t4, yC = products_emit(blk, gc)
                if nxt is not None:
                    invn = inv_emit(scn)
                    etn = qe_emit(invn)
                tree_tail(blk, t4, yC, rr, late=(blk == NBLK - 1))
                if nxt is not None:
                    gbs[nxt] = gb_emit(invn, etn)

    if not for_sim and not nc.is_finalized():
        nc.finalize()
    return nc


def _prep_inputs(x, conv_w, bn_gamma, bn_beta, bn_mean, bn_var):
    cst, win, gin = _build_consts(conv_w, bn_gamma, bn_beta, bn_mean, bn_var)
    xp = np.pad(np.asarray(x, np.float32), ((0, 0), (0, 0), (1, 1), (1, 1)),
                mode="reflect").astype(np.float16)                    # [8,64,130,130]
    in_maps = []
    for n in range(N):
        xc = np.concatenate([xp[n, :, 0:RS, :], xp[n, :, 64:64 + RS, :]], axis=0)
        xpl = np.zeros((128, RS, PL, JW), np.float16)
        xpl[:, :, 0, 0:65] = xc[:, :, 0:130:2]
        xpl[:, :, 1, 0:65] = xc[:, :, 1:130:2]
        in_maps.append({"xin": xpl, "cin": cst, "win": win, "gin": gin})
    return in_maps


def _gather(results):
    out = np.empty((N, C, HO, WO), np.float32)
    for n in range(N):
        d = np.asarray(results[n]["out"], np.float32)
        out[n, :, 0:HOC, :] = d[0:64]
        out[n, :, HOC:, :] = d[64:128]
    return out


def _enable_axon_trace():
    """Register the NTFF profile hook that this image's antenv lacks."""
    if _STATE.get("trace_hooked"):
        return
    import types
    import antenv
    from concourse import bass_utils
    mod = types.ModuleType("antenv.axon_hooks")
    mod._hook = None
    mod.set_axon_ntff_profile_hook = lambda h: setattr(mod, "_hook", h)
    mod.get_axon_ntff_profile_hook = lambda: mod._hook
    sys.modules["antenv.axon_hooks"] = mod
    antenv.axon_hooks = mod
    from trn_agent_boot.trn_boot import _ntff_profile_via_ctypes
    mod._hook = _ntff_profile_via_ctypes("/opt/axon/libaxon_pjrt.so")
    bass_utils.upload_artifacts = lambda tmpdir: tmpdir
    _STATE["trace_hooked"] = True


def run(x, conv_w, bn_gamma, bn_beta, bn_mean, bn_var, trace=False):
    from concourse.bass_utils import run_bass_kernel_spmd
    if trace:
        _enable_axon_trace()
    if "nc" not in _STATE:
        _STATE["nc"] = _build_bass()
    in_maps = _prep_inputs(x, conv_w, bn_gamma, bn_beta, bn_mean, bn_var)
    res = run_bass_kernel_spmd(_STATE["nc"], in_maps, list(range(N)), trace=trace)
    _STATE["last"] = res
    return _gather(res.results)


def kernel(x, conv_w, bn_gamma, bn_beta, bn_mean, bn_var):
    return run(x, conv_w, bn_gamma, bn_beta, bn_mean, bn_var,
               trace=bool(int(os.environ.get("KERNEL_TRACE", "0"))))


# revision 14
# speedup vs baseline: 1.0695x; 1.0658x over previous
"""Trainium2 Bass kernel for nn_Downsample_Spa: sigma-conv + gaussian unfold downsample.

Math (per batch image, one NeuronCore each; batch of 8 -> 8 cores):
  xp = reflect_pad(x)                                  # [64,130,130]
  sigma[o,p] = clamp(BN(conv3x3(xp))[o,p], 1e-4)       # at stride-2 positions p only
  graw[o,p]  = exp(-0.5*d2[o]/sigma^2 - ln64) / sigma  # /64 guards fp16 range; cancels in the ratio
  out[c,p]   = sum_o graw[o,p]*xp[c,p+off(o)] / sum_o graw[o,p]

v2 design (vs v1 baseline):
 - partitions = (row-half hh, channel c) = 128; host pre-pads (reflect), fp16, and
   parity-splits columns into TWO planes (w=2j / w=2j+1); the former third plane
   (w=2j+2) is just plane0 shifted one j-slot, so taps read plane0 at col+1.
   Input: 2.2MB/core (was 3.3MB).
 - conv: 9 accumulating fp16 matmuls per 512-position block (block-diagonal weights,
   M=18 = both row halves). sigma [18,512] in PSUM, per-BLOCK g pipeline:
   clamp (DVE tensor_scalar), fast reciprocal, ACT Square+Exp, mult -> gb bf16.
 - unfold: one-hot bf16 matmuls broadcast gb to the 128 (hh,c) partitions, 3 taps
   per PSUM rowgroup + a Srep (sum) matmul; ACT copies rowgroups to fp16 gcAll;
   all 9 tap products fp16 on DVE; pair tree ytA+ytB -> t4 -> t2 -> (gpsimd) t1 ->
   +center -> *1/S -> fp16 out DMA per block. Host converts fp32.
 - PE kept HAM-warm: ~3.4us warmup burst, per-block [Srep, 9 bcasts, next conv]
   bursts, LDWEIGHTS fillers in gaps (cold PE halves matmul throughput).
 - DMA: input chunks issued from 4 different engines in parallel; fp16 output.
"""

import os
import sys

import numpy as np

if "/opt/trn_rl_repo" not in sys.path:
    sys.path.insert(0, "/opt/trn_rl_repo")

K = 3
BN_EPS = 1e-5
SIGMA_MIN = 1e-4
GSCALE_LN = float(np.log(64.0))   # graw scaled by 1/64 (folded into exp bias)
N, C, H, W = 8, 64, 128, 128
HO = WO = 64
HH = 2
RS = 65                  # padded-row slots per partition-half
HOC = 32
NBLK = 4
BR = HOC // NBLK         # 8 output rows per block
NPOS = BR * WO           # 512
PL = 2                   # x col-parity planes: w=2j / w=2j+1
JW = 66                  # j slots per plane (65 used, 66 for alignment)
CR = 17                  # rows per DMA chunk tile (16 + 1 overlap)

# f32 consts tensor columns
_D2 = 0                  # -0.5*d2[o] per (hh,o)
_BC = 1                  # bn_bias - sigma_min
_LB = 2                  # exp bias: constant -ln(64) per partition
_NCC = 3

_STATE = {}


def _build_consts(conv_w, bn_gamma, bn_beta, bn_mean, bn_var):
    s = (bn_gamma / np.sqrt(bn_var + BN_EPS)).astype(np.float32)
    wf = conv_w.astype(np.float32) * s[:, None, None, None]           # [9,64,3,3]
    bias = (bn_beta - bn_mean * s).astype(np.float32)

    cst = np.zeros((18, _NCC), np.float32)
    d2 = np.array([(kk // 3 - 1) ** 2 + (kk % 3 - 1) ** 2 for kk in range(9)], np.float32)
    for hh in range(HH):
        cst[hh * 9:hh * 9 + 9, _D2] = -0.5 * d2
        cst[hh * 9:hh * 9 + 9, _BC] = bias - SIGMA_MIN
        cst[hh * 9:hh * 9 + 9, _LB] = -GSCALE_LN

    # conv weights, block-diagonal per tap: win[k=hh*64+c, tap*18 + hh*9+o]
    win = np.zeros((128, 9 * 18), np.float16)
    for tap in range(9):
        i, j = tap // 3, tap % 3
        for hh in range(HH):
            win[hh * 64:hh * 64 + 64, tap * 18 + hh * 9:tap * 18 + hh * 9 + 9] = \
                wf[:, :, i, j].T.astype(np.float16)

    # one-hot / ones broadcast weights: gin[k=hh*9+o, tap*128 + hh*64+c]
    import ml_dtypes
    gin = np.zeros((18, 10 * 128), ml_dtypes.bfloat16)
    for hh in range(HH):
        gin[hh * 9:hh * 9 + 9, 9 * 128 + hh * 64:9 * 128 + hh * 64 + 64] = 1.0
        for tap in range(9):
            gin[hh * 9 + tap, tap * 128 + hh * 64:tap * 128 + hh * 64 + 64] = 1.0
    return cst, win, gin


def _build_bass(for_sim=False):
    import concourse.bass as bass
    import concourse.tile as tile
    from concourse import mybir

    f32 = mybir.dt.float32
    f16 = mybir.dt.float16
    bf16 = mybir.dt.bfloat16
    MULT = mybir.AluOpType.mult
    ADD = mybir.AluOpType.add
    MAX = mybir.AluOpType.max
    AF = mybir.ActivationFunctionType

    if for_sim:
        nc = bass.Bass("TRN2", target_bir_lowering=False, detect_race_conditions=False)
    else:
        from concourse import bacc
        nc = bacc.Bacc()
    xin = nc.dram_tensor("xin", [128, RS, PL, JW], f16, kind="ExternalInput")
    cin = nc.dram_tensor("cin", [18, _NCC], f32, kind="ExternalInput")
    win = nc.dram_tensor("win", [128, 9 * 18], f16, kind="ExternalInput")
    gin = nc.dram_tensor("gin", [18, 10 * 128], bf16, kind="ExternalInput")
    out = nc.dram_tensor("out", [128, HOC, WO], f16, kind="ExternalOutput")

    with tile.TileContext(nc) as tc:
        from contextlib import ExitStack
        with ExitStack() as ctx:
            big = ctx.enter_context(tc.tile_pool(name="big", bufs=1))
            gsb = ctx.enter_context(tc.tile_pool(name="gsb", bufs=3))
            y_p = ctx.enter_context(tc.tile_pool(name="y", bufs=3))
            ps_s = ctx.enter_context(tc.tile_pool(name="ps_s", bufs=2, space="PSUM"))
            ps_g = ctx.enter_context(tc.tile_pool(name="ps_g", bufs=2, space="PSUM"))

            # warm-up operand: memset scratch, so the PE can start before any DMA
            scr = big.tile([128, 162], f16)
            nc.vector.memset(scr[:], 0.0)

            # ---- input DMAs: hw queues only (sync/scalar); gpsimd's software DGE
            # has multi-us latency. Block-0 chunk first, then the small consts ----
            ws = big.tile([128, 9 * 18], f16)
            cs = big.tile([18, _NCC], f32)
            gs = big.tile([18, 10 * 128], bf16)
            xsk = []
            for blk in range(NBLK):
                xs = big.tile([128, CR, PL, JW], f16, tag=f"xs{blk}")
                xsk.append(xs)
            nc.sync.dma_start(out=xsk[0][:], in_=xin[:, 0:CR, :, :])
            nc.sync.dma_start(out=ws[:], in_=win[:])
            nc.sync.dma_start(out=cs[:], in_=cin[:])
            nc.sync.dma_start(out=gs[:], in_=gin[:])
            nc.sync.dma_start(out=xsk[1][:], in_=xin[:, 16:16 + CR, :, :])
            nc.scalar.dma_start(out=xsk[2][:], in_=xin[:, 32:32 + CR, :, :])
            nc.scalar.dma_start(out=xsk[3][:], in_=xin[:, 48:48 + CR, :, :])

            def xtap(tap, blk):
                # [128, 8, 64] fp16 step-1 view for conv tap (i,j) in block blk
                i, j = tap // 3, tap % 3
                if j < 2:
                    return xsk[blk][:, i:i + 2 * BR - 1:2, j, 0:WO]
                return xsk[blk][:, i:i + 2 * BR - 1:2, 0, 1:WO + 1]

            def xpair(i, blk):
                # [128, 2, 8, 64] planes (w=2j, 2j+1) for tap row i
                return xsk[blk][:, i:i + 2 * BR - 1:2, 0:PL, 0:WO].transpose([0, 2, 1, 3])

            # ---- PE warm-up: ~3.4us of matmuls on the memset scratch while input
            # DMA lands, so HAM un-throttles before the conv ----
            wu = ps_s.tile([18, BR, WO], f32, tag="sig")
            for _ in range(26):
                nc.tensor.matmul(wu[:, 0:3, 0:54], scr[:, 0:18], scr[:, 0:162],
                                 start=True, stop=True)

            def warm_fill(n):
                # real matmuls on the scratch tile: only array-active work counts
                # toward HAM's warm window (LDWEIGHTS does not)
                wuf = ps_s.tile([18, BR, WO], f32, tag="sig")
                for _ in range(n):
                    nc.tensor.matmul(wuf[:, 0:3, 0:54], scr[:, 0:18], scr[:, 0:162],
                                     start=True, stop=True)

            def conv_mm(blk):
                sig = ps_s.tile([18, BR, WO], f32, tag="sig")
                for tap in range(9):
                    nc.tensor.matmul(
                        sig[:],
                        ws[:, tap * 18:(tap + 1) * 18],
                        xtap(tap, blk),
                        start=(tap == 0), stop=(tap == 8),
                    )
                return sig

            def clamp_emit(sig):
                sc = gsb.tile([18, BR, WO], f32, tag="sc")
                nc.vector.tensor_scalar(out=sc[:], in0=sig[:],
                                        scalar1=cs[:, _BC:_BC + 1],
                                        scalar2=float(SIGMA_MIN),
                                        op0=ADD, op1=MAX)
                return sc

            def inv_emit(sc):
                inv = gsb.tile([18, BR, WO], f32, tag="inv")
                nc.vector.reciprocal_approx_fast(out=inv[:], in_=sc[:])
                return inv

            def qe_emit(inv):
                qt = gsb.tile([18, BR, WO], f32, tag="qt")
                nc.scalar.activation(out=qt[:], in_=inv[:], func=AF.Square)
                et = gsb.tile([18, BR, WO], f32, tag="et")
                nc.scalar.activation(out=et[:], in_=qt[:], func=AF.Exp,
                                     scale=cs[:, _D2:_D2 + 1],
                                     bias=cs[:, _LB:_LB + 1])
                return et

            def gb_emit(inv, et):
                gb = gsb.tile([18, BR, WO], bf16, tag="gb")
                nc.vector.tensor_tensor(out=gb[:], in0=et[:], in1=inv[:], op=MULT)
                return gb

            def burst_mm(blk, gb):
                # PE burst: Srep + 9 broadcast matmuls, back-to-back with the next
                # conv block so each block's PE stretch re-triggers the HAM window.
                gbf = gb[:]  # [18, 8, 64]; matmul checks free_size only
                Srep = ps_s.tile([128, BR, WO], f32, tag="sig")
                nc.tensor.matmul(Srep[:], gs[:, 9 * 128:10 * 128], gbf,
                                 start=True, stop=True)
                groups = []
                for g in range(3):  # rowgroups: taps (0,1,2), (3,4,5), (6,7,8)
                    g3 = ps_g.tile([128, 3, BR, WO], f32, tag="grep")
                    for k in range(3):
                        tap = 3 * g + k
                        nc.tensor.matmul(g3[:, k], gs[:, tap * 128:(tap + 1) * 128],
                                         gbf, start=True, stop=True)
                    groups.append(g3)
                return Srep, groups

            def copies_emit(groups):
                gc = y_p.tile([128, 9, BR, WO], f16, tag="gc")
                for g in range(3):
                    nc.scalar.activation(out=gc[:, 3 * g:3 * g + 3],
                                         in_=groups[g][:], func=AF.Copy)
                return gc

            def products_emit(blk, gc):
                # products, all fp16: ytA = [r0j0, r0j1, r0j2, r1j0],
                # ytB = [r2j0, r2j1, r2j2, r1j2], yC = center
                ytA = y_p.tile([128, 4, BR, WO], f16, tag="ytA")
                ytB = y_p.tile([128, 4, BR, WO], f16, tag="ytB")
                yC = y_p.tile([128, BR, WO], f16, tag="yC")
                nc.vector.tensor_tensor(out=ytA[:, 0:2], in0=xpair(0, blk),
                                        in1=gc[:, 0:2], op=MULT)
                nc.vector.tensor_tensor(out=ytA[:, 2], in0=xtap(2, blk),
                                        in1=gc[:, 2], op=MULT)
                nc.vector.tensor_tensor(out=ytA[:, 3], in0=xtap(3, blk),
                                        in1=gc[:, 3], op=MULT)
                nc.vector.tensor_tensor(out=yC[:], in0=xtap(4, blk),
                                        in1=gc[:, 4], op=MULT)
                nc.vector.tensor_tensor(out=ytB[:, 3], in0=xtap(5, blk),
                                        in1=gc[:, 5], op=MULT)
                nc.vector.tensor_tensor(out=ytB[:, 0:2], in0=xpair(2, blk),
                                        in1=gc[:, 6:8], op=MULT)
                nc.vector.tensor_tensor(out=ytB[:, 2], in0=xtap(8, blk),
                                        in1=gc[:, 8], op=MULT)
                t4 = y_p.tile([128, 4, BR, WO], f16, tag="t4")
                nc.vector.tensor_tensor(out=t4[:], in0=ytA[:], in1=ytB[:], op=ADD)
                return t4, yC

            def tree_tail(blk, t4, yC, rr, late):
                t2 = y_p.tile([128, 2, BR, WO], f16, tag="t2")
                nc.vector.tensor_tensor(out=t2[:], in0=t4[:, 0:2], in1=t4[:, 2:4], op=ADD)
                eng1 = nc.vector if late else nc.gpsimd
                t1 = y_p.tile([128, BR, WO], f16, tag="t1")
                eng1.tensor_tensor(out=t1[:], in0=t2[:, 0], in1=t2[:, 1], op=ADD)
                tC = y_p.tile([128, BR, WO], f16, tag="tC")
                eng1.tensor_tensor(out=tC[:], in0=t1[:], in1=yC[:], op=ADD)
                o16 = y_p.tile([128, BR, WO], f16, tag="o16")
                eng1.tensor_tensor(out=o16[:], in0=tC[:], in1=rr[:], op=MULT)
                nc.sync.dma_start(out=out[:, BR * blk:BR * (blk + 1), :], in_=o16[:])

            # ---- schedule: lead-in (conv0, conv1 + their g-pipes), then per-block
            # [PE burst + next conv] / [copies] / [products] staged for overlap ----
            sig0 = conv_mm(0)
            sc0 = clamp_emit(sig0)
            inv0 = inv_emit(sc0)
            et0 = qe_emit(inv0)
            sig1 = conv_mm(1)
            sc1 = clamp_emit(sig1)
            inv1 = inv_emit(sc1)
            et1 = qe_emit(inv1)
            gbs = [gb_emit(inv0, et0), gb_emit(inv1, et1), None, None]
            warm_fill(10)

            for blk in range(NBLK):
                nxt = blk + 2 if blk + 2 < NBLK else None
                Srep, groups = burst_mm(blk, gbs[blk])
                rr = y_p.tile([128, BR, WO], f32, tag="rr")
                nc.vector.reciprocal_approx_fast(out=rr[:], in_=Srep[:])
                gc = copies_emit(groups)
                if nxt is not None:
                    sign = conv_mm(nxt)
                    scn = clamp_emit(sign)
                else:
                    warm_fill(8)
                t4, yC = products_emit(blk, gc)
                if nxt is not None:
                    invn = inv_emit(scn)
                    etn = qe_emit(invn)
                tree_tail(blk, t4, yC, rr, late=(blk == NBLK - 1))
                if nxt is not None:
                    gbs[nxt] = gb_emit(invn, etn)

    if not for_sim and not nc.is_finalized():
        nc.finalize()
    return nc


def _prep_inputs(x, conv_w, bn_gamma, bn_beta, bn_mean, bn_var):
    cst, win, gin = _build_consts(conv_w, bn_gamma, bn_beta, bn_mean, bn_var)
    xp = np.pad(np.asarray(x, np.float32), ((0, 0), (0, 0), (1, 1), (1, 1)),
                mode="reflect").astype(np.float16)                    # [8,64,130,130]
    in_maps = []
    for n in range(N):
        xc = np.concatenate([xp[n, :, 0:RS, :], xp[n, :, 64:64 + RS, :]], axis=0)
        xpl = np.zeros((128, RS, PL, JW), np.float16)
        xpl[:, :, 0, 0:65] = xc[:, :, 0:130:2]
        xpl[:, :, 1, 0:65] = xc[:, :, 1:130:2]
        in_maps.append({"xin": xpl, "cin": cst, "win": win, "gin": gin})
    return in_maps


def _gather(results):
    out = np.empty((N, C, HO, WO), np.float32)
    for n in range(N):
        d = np.asarray(results[n]["out"], np.float32)
        out[n, :, 0:HOC, :] = d[0:64]
        out[n, :, HOC:, :] = d[64:128]
    return out


def _enable_axon_trace():
    """Register the NTFF profile hook that this image's antenv lacks."""
    if _STATE.get("trace_hooked"):
        return
    import types
    import antenv
    from concourse import bass_utils
    mod = types.ModuleType("antenv.axon_hooks")
    mod._hook = None
    mod.set_axon_ntff_profile_hook = lambda h: setattr(mod, "_hook", h)
    mod.get_axon_ntff_profile_hook = lambda: mod._hook
    sys.modules["antenv.axon_hooks"] = mod
    antenv.axon_hooks = mod
    from trn_agent_boot.trn_boot import _ntff_profile_via_ctypes
    mod._hook = _ntff_profile_via_ctypes("/opt/axon/libaxon_pjrt.so")
    bass_utils.upload_artifacts = lambda tmpdir: tmpdir
    _STATE["trace_hooked"] = True


def run(x, conv_w, bn_gamma, bn_beta, bn_mean, bn_var, trace=False):
    from concourse.bass_utils import run_bass_kernel_spmd
    if trace:
        _enable_axon_trace()
    if "nc" not in _STATE:
        _STATE["nc"] = _build_bass()
    in_maps = _prep_inputs(x, conv_w, bn_gamma, bn_beta, bn_mean, bn_var)
    res = run_bass_kernel_spmd(_STATE["nc"], in_maps, list(range(N)), trace=trace)
    _STATE["last"] = res
    return _gather(res.results)


def kernel(x, conv_w, bn_gamma, bn_beta, bn_mean, bn_var):
    return run(x, conv_w, bn_gamma, bn_beta, bn_mean, bn_var,
               trace=bool(int(os.environ.get("KERNEL_TRACE", "0"))))
